# revision 1
# baseline (speedup 1.0000x reference)
"""Fused multi-head-attention (full-width variant) for 8 TRN2 NeuronCores.

Strategy: pure data-parallel over batch (B=8 -> one batch per core).
Per core, with everything in "feature-on-partition" transposed layouts:
  qT/kT = Wq/Wk @ xT               (f32r matmuls, tf32-precision logit path;
                                    the x8 energy scale is folded into Wq on host)
  E     = q @ k.T                  (f32r, fp32 PSUM accum; already x8)
  P     = softmax rows via ACT exp (bias=-rowmax via negated reduce, accum rowsum)
  PT    = DMA-xbar transpose of P  (bf16)
  outT  = v.T @ PT                 (bf16 matmuls)
  yT    = Wo @ outT                (f32r)
Host transposes x/W in, yT out.
"""
import sys

sys.path.insert(0, "/opt/trn_rl_repo")

import numpy as np

import concourse.bass as bass  # noqa: F401
import concourse.tile as tile
from concourse import bacc, mybir

F32 = mybir.dt.float32
F32R = mybir.dt.float32r
BF16 = mybir.dt.bfloat16
AX = mybir.AxisListType.X
MAX = mybir.AluOpType.max

B = 8
E = 768
N = 2048
EC = E // 128      # 6 feature chunks
NT = N // 128      # 16 token chunks
NBLK = N // 512    # 4 blocks of 512 tokens
SCALE = 8.0        # sqrt(head_dim); reference multiplies by it

_CACHE = {}


def _build():
    nc = bacc.Bacc("TRN2", target_bir_lowering=False, debug=False, num_devices=B)

    xT_d = nc.dram_tensor("xT", [E, N], F32R, kind="ExternalInput")
    wq_d = nc.dram_tensor("wq", [E, E], F32R, kind="ExternalInput")
    wk_d = nc.dram_tensor("wk", [E, E], F32R, kind="ExternalInput")
    wv_d = nc.dram_tensor("wv", [E, E], F32R, kind="ExternalInput")
    wo_d = nc.dram_tensor("wo", [E, E], F32R, kind="ExternalInput")
    yT_d = nc.dram_tensor("yT", [E, N], F32, kind="ExternalOutput")
    # Tiny per-tile stats dump. Its real job: a plain HWDGE DMA queued before
    # every dma_start_transpose — two xbar transposes back-to-back on the sync
    # queue with no intervening plain DMA produce doubled output values
    # (observed on HW; the plain transfer forces the xbar-mode transition).
    snk_d = nc.dram_tensor("snk", [NT, 128, 8], F32, kind="ExternalOutput")

    xT_r = xT_d.rearrange("(c p) n -> p c n", p=128)
    wq_r = wq_d.rearrange("(c p) f -> p c f", p=128)
    wk_r = wk_d.rearrange("(c p) f -> p c f", p=128)
    wv_r = wv_d.rearrange("(c p) f -> p c f", p=128)
    wo_r = wo_d.rearrange("(c p) f -> p c f", p=128)
    yT_r = yT_d.rearrange("(c p) n -> p c n", p=128)

    with tile.TileContext(nc) as tc:
        with tc.tile_pool(name="kT", bufs=1) as ktp, \
             tc.tile_pool(name="qT", bufs=1) as qtp, \
             tc.tile_pool(name="vv", bufs=1) as vvp:
            kT = ktp.tile([128, EC, N], F32R)   # 48 KB/partition
            qT = qtp.tile([128, EC, N], F32R)   # 48
            v = vvp.tile([128, NT, E], BF16)    # 24

            # ---------------- stage B: projections ----------------
            with tc.tile_pool(name="xt", bufs=1) as xtp, \
                 tc.tile_pool(name="wp", bufs=2) as wpp, \
                 tc.tile_pool(name="psb", bufs=8, space="PSUM") as psb:
                # PE warm-up during the initial input-DMA window: dummy
                # matmuls push the HAM activity window so the first real
                # matmuls run at 2.4 GHz instead of 1.2 GHz
                wrm = xtp.tile([128, 512], BF16, tag="wrm")
                nc.vector.memset(wrm[:], 0.0)
                wps = psb.tile([128, 512], F32, tag="ps")
                for _w in range(18):
                    nc.tensor.matmul(
                        wrm_mm := wps[:],
                        lhsT=wrm[:, 0:128],
                        rhs=wrm[:],
                        start=True,
                        stop=True,
                    )
                xT = xtp.tile([128, EC, N], F32R)  # 48
                wk_t = wpp.tile([128, EC, E], F32R, tag="w")  # 18 x2
                # DMA order tuned for earliest PE start: xT-nb0, wk-f0,
                # xT-nb1, rest of wk, xT-nb2/3, wq, wv (projection order is
                # kT -> qT -> v, so attention can start after kT+qT(nb0))
                nc.sync.dma_start(xT[:, :, 0:512], xT_r[:, :, 0:512])
                nc.sync.dma_start(wk_t[:, :, 0:128], wk_r[:, :, 0:128])
                nc.sync.dma_start(xT[:, :, 512:1024], xT_r[:, :, 512:1024])
                for f in range(1, EC):
                    nc.sync.dma_start(
                        wk_t[:, :, f * 128:(f + 1) * 128],
                        wk_r[:, :, f * 128:(f + 1) * 128],
                    )
                nc.sync.dma_start(xT[:, :, 1024:1536], xT_r[:, :, 1024:1536])
                nc.sync.dma_start(xT[:, :, 1536:2048], xT_r[:, :, 1536:2048])
                wq_t = wpp.tile([128, EC, E], F32R, tag="w")
                nc.sync.dma_start(wq_t[:], wq_r[:])

                # kT = Wk @ xT   (nb-outer: group nb needs only xT block nb)
                for nb in range(NBLK):
                    for f in range(EC):
                        ps = psb.tile([128, 512], F32, tag="ps")
                        for e in range(EC):
                            nc.tensor.matmul(
                                ps[:],
                                lhsT=wk_t[:, e, f * 128:(f + 1) * 128],
                                rhs=xT[:, e, nb * 512:(nb + 1) * 512],
                                start=(e == 0),
                                stop=(e == EC - 1),
                            )
                        nc.scalar.copy(kT[:, f, nb * 512:(nb + 1) * 512], ps[:])

                # qT = Wq @ xT   (nb-outer so attention can start at nb=0)
                for nb in range(NBLK):
                    for f in range(EC):
                        ps = psb.tile([128, 512], F32, tag="ps")
                        for e in range(EC):
                            nc.tensor.matmul(
                                ps[:],
                                lhsT=wq_t[:, e, f * 128:(f + 1) * 128],
                                rhs=xT[:, e, nb * 512:(nb + 1) * 512],
                                start=(e == 0),
                                stop=(e == EC - 1),
                            )
                        nc.scalar.copy(qT[:, f, nb * 512:(nb + 1) * 512], ps[:])

                # v (natural layout, bf16) = x @ Wv.T  (wv reuses wk's slot)
                wv_t = wpp.tile([128, EC, E], F32R, tag="w")
                nc.sync.dma_start(wv_t[:], wv_r[:])
                for t in range(NT):
                    for flo, fhi in ((0, 512), (512, 768)):
                        ps = psb.tile([128, 512], F32, tag="ps")
                        for e in range(EC):
                            nc.tensor.matmul(
                                ps[:, :fhi - flo],
                                lhsT=xT[:, e, t * 128:(t + 1) * 128],
                                rhs=wv_t[:, e, flo:fhi],
                                start=(e == 0),
                                stop=(e == EC - 1),
                            )
                        nc.scalar.copy(v[:, t, flo:fhi], ps[:, :fhi - flo])

            # ---------------- stage C/D/E: attention ----------------
            with tc.tile_pool(name="wo", bufs=1) as wop, \
                 tc.tile_pool(name="pp", bufs=5) as ppp, \
                 tc.tile_pool(name="pt", bufs=2) as ptp, \
                 tc.tile_pool(name="ot", bufs=1) as otp, \
                 tc.tile_pool(name="yt", bufs=4) as ytp, \
                 tc.tile_pool(name="st", bufs=8) as stp, \
                 tc.tile_pool(name="pse", bufs=6, space="PSUM") as pse, \
                 tc.tile_pool(name="psm", bufs=2, space="PSUM") as psm:
                wo_t = wop.tile([128, EC, E], F32R)  # 18
                nc.sync.dma_start(wo_t[:], wo_r[:])

                for ib in range(NBLK):
                    pt_blk = ptp.tile([128, NT, 512], BF16)  # 16 x2
                    for t4 in range(4):
                        i = ib * 4 + t4
                        stats = stp.tile([128, 8], F32, tag="stats")
                        e_tiles = []
                        for jb in range(NBLK):
                            pe = pse.tile([128, 512], F32)
                            for d in range(EC):
                                nc.tensor.matmul(
                                    pe[:],
                                    lhsT=qT[:, d, i * 128:(i + 1) * 128],
                                    rhs=kT[:, d, jb * 512:(jb + 1) * 512],
                                    start=(d == 0),
                                    stop=(d == EC - 1),
                                )
                            nc.vector.tensor_reduce(
                                stats[:, jb:jb + 1], pe[:], axis=AX, op=MAX
                            )
                            e_tiles.append(pe)
                        nmax = stp.tile([128, 1], F32, tag="nmax")
                        nc.vector.tensor_reduce(
                            nmax[:], stats[:, 0:4], axis=AX, op=MAX, negate=True)

                        p_t = ppp.tile([128, N], BF16)  # 4 x2
                        for jb in range(NBLK):
                            nc.scalar.activation(
                                p_t[:, jb * 512:(jb + 1) * 512],
                                e_tiles[jb][:],
                                func=mybir.ActivationFunctionType.Exp,
                                bias=nmax[:],
                                scale=1.0,
                                accum_out=stats[:, 4 + jb:5 + jb],
                            )
                        rs = stp.tile([128, 1], F32, tag="rs")
                        nc.vector.tensor_reduce(
                            rs[:], stats[:, 4:8], axis=AX, op=mybir.AluOpType.add
                        )
                        rcp = stp.tile([128, 1], F32, tag="rcp")
                        nc.vector.reciprocal(rcp[:], rs[:])
                        nc.vector.tensor_scalar_mul(p_t[:], p_t[:], rcp[:])
                        # ALL transposes on one HWDGE queue, each preceded by
                        # a plain guard DMA: concurrent xbar transposes (even on
                        # different queues) corrupt results — xbar state is
                        # per-core global
                        nc.sync.dma_start(snk_d[i], stats[:])
                        nc.sync.dma_start_transpose(
                            pt_blk[:, :, t4 * 128:(t4 + 1) * 128], p_t[:]
                        )

                    # outT = v.T @ PT (bf16), split into i-halves so the first
                    # half starts after transposes t4=0,1 only
                    ot_blk = otp.tile([128, EC, 512], F32R)  # 12
                    for lo, hi in ((0, 256), (256, 512)):
                        for d in range(EC):
                            po = psm.tile([128, 256], F32, tag="mm")
                            for jc in range(NT):
                                nc.tensor.matmul(
                                    po[:],
                                    lhsT=v[:, jc, d * 128:(d + 1) * 128],
                                    rhs=pt_blk[:, jc, lo:hi],
                                    start=(jc == 0),
                                    stop=(jc == NT - 1),
                                )
                            nc.vector.tensor_copy(ot_blk[:, d, lo:hi], po[:])

                        # yT = Wo @ outT  (f32r) for this i-half
                        for f in range(EC):
                            py = psm.tile([128, 256], F32, tag="mm")
                            for e in range(EC):
                                nc.tensor.matmul(
                                    py[:],
                                    lhsT=wo_t[:, e, f * 128:(f + 1) * 128],
                                    rhs=ot_blk[:, e, lo:hi],
                                    start=(e == 0),
                                    stop=(e == EC - 1),
                                )
                            yt = ytp.tile([128, 256], F32)
                            nc.vector.tensor_copy(yt[:], py[:])
                            nc.sync.dma_start(
                                yT_r[:, f, ib * 512 + lo:ib * 512 + hi], yt[:]
                            )

    nc.finalize()
    return nc


def _get_nc():
    if "nc" not in _CACHE:
        _CACHE["nc"] = _build()
    return _CACHE["nc"]


def kernel(x, Wq, Wk, Wv, Wo, _run_kwargs=None):
    from concourse.bass_utils import run_bass_kernel_spmd

    x = np.asarray(x, dtype=np.float32)
    # fold the sqrt(head_dim) energy scale into Wq (exact: power of 2)
    wq = np.ascontiguousarray(np.asarray(Wq, dtype=np.float32).T * SCALE)
    wk = np.ascontiguousarray(np.asarray(Wk, dtype=np.float32).T)
    wv = np.ascontiguousarray(np.asarray(Wv, dtype=np.float32).T)
    wo = np.ascontiguousarray(np.asarray(Wo, dtype=np.float32).T)

    nc = _get_nc()
    in_maps = [
        {
            "xT": np.ascontiguousarray(x[b].T),
            "wq": wq,
            "wk": wk,
            "wv": wv,
            "wo": wo,
        }
        for b in range(B)
    ]
    res = run_bass_kernel_spmd(nc, in_maps, list(range(B)), **(_run_kwargs or {}))
    out = np.stack([res.results[b]["yT"].T for b in range(B)])
    if _run_kwargs:
        _CACHE["last_results"] = res
    return np.ascontiguousarray(out, dtype=np.float32)

